# revision 15
# baseline (speedup 1.0000x reference)
"""DistMult metapath scoring kernel for Trainium2 (8 NeuronCores).

Math (from the reference): every output group reduces to
    score_i = emb_h[idx_i] @ c        with c = K @ s a fixed [d] vector per group
where s is a sum of gathered embedding rows:
    pos0: idx=ei0[0]         s=sum emb_A[ei0[1]]     c=K0@s
    pos1: idx=ei1[0]         s=sum emb_B[ei1[1]]     c=K1@s
    nh0:  idx=nh0.flat       s=sum emb_A[nh0[:,0]]   c=16*K0@s
    nh1:  idx=nh1.flat       s=sum emb_A[nh1[:,0]]   c=16*K1@s
    nt0:  idx=nt0[:,0] (x16) s=sum emb_A[nt0.flat]   c=K0@s
    nt1:  idx=nt1[:,0] (x16) s=sum emb_B[nt1.flat]   c=K1@s

Device computation (node-parallel SPMD on 8 cores, two launches, no
gathers and no collectives -- a [128,6] collective costs ~90us in barrier +
AllReduce latency, far more than a host combine of 8 partial vectors):
  Launch 1 (sums): each sum is a count-weighted dense reduction
     s = emb.T @ counts, counts[n] = multiplicity of node n in the index set
     (host bincount -- index-side preprocessing, same family as the previous
     version's bucketing/take maps). Nodes are sharded 8 ways; each core
     emits partial sums sT_k [128(d), 6] via PE matmuls (emb tile as
     weights, counts as rhs, f32 PSUM accumulation).
  Host: gsum = sum_k sT_k  (unshard of the sum-sharded partials, 768 floats)
  Launch 2 (projection): head c_g = K_g @ s_g on PE with host-pretransposed
     K, x16 scaling for the nh groups, cast bf16 -> C_A [128,5], C_B [128,1];
     then dense projection over the core's node slice q = C.T @ embT
     (d-major table, 512-col PE matmuls) -> per-node scores qA [5, nodes],
     qB [1, nodes].
Host glue: per-edge scores are reads of q (out_i = q[col, idx_i]) -- the
final np.take / x16 repeat expansion is host-side, exactly like the previous
version's take maps (which already expanded nt x16 and inverse-permuted all
device scores host-side). Tables are fed in bf16 (rel tol is 2e-2; measured
end-to-end error ~3e-3); all device accumulation is f32.
"""

import sys
from contextlib import ExitStack

import numpy as np

sys.path.insert(0, "/opt/trn_rl_repo")

import concourse.bass as bass
from concourse import bacc, mybir
from concourse.bass_utils import run_bass_kernel_spmd

D = 128
E = 50000
S = 16
NA = 100000
NB = 50000
NCORES = 8

SLA = NA // NCORES          # 12500 A-nodes per core
SLB = NB // NCORES          # 6250 B-nodes per core
TA = (SLA + 127) // 128     # 98 tiles
TB = (SLB + 127) // 128     # 49 tiles
PLA = TA * 128              # 12544 padded
PLB = TB * 128              # 6272 padded
GRP = 14                    # node tiles per DMA chunk (load/compute overlap)

F32 = mybir.dt.float32
BF16 = mybir.dt.bfloat16


def _chunks(n, c):
    out = []
    j = 0
    while j < n:
        out.append((j, min(c, n - j)))
        j += c
    return out


def build_sums() -> bass.Bass:
    nc = bacc.Bacc(None, target_bir_lowering=False)
    tnA = nc.dram_tensor("tnA", [128, TA * D], BF16, kind="ExternalInput")
    tnB = nc.dram_tensor("tnB", [128, TB * D], BF16, kind="ExternalInput")
    cntA = nc.dram_tensor("cntA", [128, TA * 4], BF16, kind="ExternalInput")
    cntB = nc.dram_tensor("cntB", [128, TB * 2], BF16, kind="ExternalInput")
    part = nc.dram_tensor("part", [D, 6], F32, kind="ExternalOutput")

    with ExitStack() as ctx:
        from concourse.tile import TileContext

        tc = ctx.enter_context(TileContext(nc))
        sing = ctx.enter_context(tc.tile_pool(name="sing", bufs=1))
        spp = ctx.enter_context(tc.tile_pool(name="sp", bufs=1, space="PSUM"))

        # chunked loads on separate queues (counts: Pool SWDGE, table A: SP,
        # table B: ACT) so the first matmul starts ~1.5us in, overlapped with
        # the rest of the load stream
        cA_s = sing.tile([128, TA * 4], BF16, tag="cA")
        nc.gpsimd.dma_start(out=cA_s[:, :], in_=cntA[:, :])
        cB_s = sing.tile([128, TB * 2], BF16, tag="cB")
        nc.gpsimd.dma_start(out=cB_s[:, :], in_=cntB[:, :])
        tnA_t, tnB_t = [], []
        for g, (t0, nt) in enumerate(_chunks(TA, GRP)):
            t = sing.tile([128, nt * D], BF16, tag=f"tnA{g}")
            nc.sync.dma_start(out=t[:, :], in_=tnA[:, t0 * D : (t0 + nt) * D])
            tnA_t.append((t0, nt, t))
        for g, (t0, nt) in enumerate(_chunks(TB, GRP)):
            t = sing.tile([128, nt * D], BF16, tag=f"tnB{g}")
            nc.scalar.dma_start(out=t[:, :], in_=tnB[:, t0 * D : (t0 + nt) * D])
            tnB_t.append((t0, nt, t))

        psA = spp.tile([128, 4], F32, tag="psA")
        for t0, nt, tile in tnA_t:
            for i in range(nt):
                t = t0 + i
                nc.tensor.matmul(
                    out=psA[:, :],
                    lhsT=tile[:, i * D : (i + 1) * D],
                    rhs=cA_s[:, t * 4 : (t + 1) * 4],
                    start=(t == 0),
                    stop=(t == TA - 1),
                )
        psB = spp.tile([128, 2], F32, tag="psB")
        for t0, nt, tile in tnB_t:
            for i in range(nt):
                t = t0 + i
                nc.tensor.matmul(
                    out=psB[:, :],
                    lhsT=tile[:, i * D : (i + 1) * D],
                    rhs=cB_s[:, t * 2 : (t + 1) * 2],
                    start=(t == 0),
                    stop=(t == TB - 1),
                )
        sb6 = sing.tile([128, 6], F32, tag="sb6")
        nc.vector.tensor_copy(sb6[:, 0:4], psA[:, :])
        nc.vector.tensor_copy(sb6[:, 4:6], psB[:, :])
        nc.sync.dma_start(out=part[:, :], in_=sb6[:, :])

    nc.compile()
    return nc


def build_proj() -> bass.Bass:
    nc = bacc.Bacc(None, target_bir_lowering=False)
    ttA = nc.dram_tensor("ttA", [128, PLA], BF16, kind="ExternalInput")
    ttB = nc.dram_tensor("ttB", [128, PLB], BF16, kind="ExternalInput")
    relT = nc.dram_tensor("relT", [2, D, D], F32, kind="ExternalInput")
    gsum = nc.dram_tensor("gsum", [D, 6], F32, kind="ExternalInput")
    qA = nc.dram_tensor("qA", [5, PLA], F32, kind="ExternalOutput")
    qB = nc.dram_tensor("qB", [1, PLB], F32, kind="ExternalOutput")

    with ExitStack() as ctx:
        from concourse.tile import TileContext

        tc = ctx.enter_context(TileContext(nc))
        sing = ctx.enter_context(tc.tile_pool(name="sing", bufs=1))
        stg = ctx.enter_context(tc.tile_pool(name="stg", bufs=3))
        qpp = ctx.enter_context(tc.tile_pool(name="qp", bufs=2, space="PSUM"))

        # head inputs on the Pool queue (small), tables chunked on SP/ACT
        # queues with one tile per 2048-col chunk so matmuls chase the loads
        sT = sing.tile([128, 6], F32, tag="sT")
        nc.gpsimd.dma_start(out=sT[:, :], in_=gsum[:, :])
        kt = []
        for m in range(2):
            k_s = sing.tile([128, 128], F32, tag=f"kt{m}")
            nc.gpsimd.dma_start(out=k_s[:, :], in_=relT[m, :, :])
            kt.append(k_s)

        def load_chunked(dram, total, eng, tag):
            tiles = {}
            for j, n in _chunks(total, 2048):
                t = sing.tile([128, n], BF16, tag=f"{tag}{j}")
                eng.dma_start(out=t[:, :], in_=dram[:, j : j + n])
                tiles[j] = t
            return tiles

        ttA_s = load_chunked(ttA, PLA, nc.sync, "ttA")
        ttB_s = load_chunked(ttB, PLB, nc.scalar, "ttB")

        # head: sT columns 0:s0 1:h0 2:t0 3:h1 4:s1 5:t1
        # K0 -> (s0,h0,t0) = (c0, c2/16, c4); K1 -> (h1,s1,t1) = (c3/16, c1, c5)
        # two 512-col slots of one psum tile = different banks, no group clash
        cp = qpp.tile([128, 2048], F32, tag="q")
        nc.tensor.matmul(
            out=cp[:, 0:3], lhsT=kt[0][:, :], rhs=sT[:, 0:3], start=True, stop=True
        )
        nc.tensor.matmul(
            out=cp[:, 512:515], lhsT=kt[1][:, :], rhs=sT[:, 3:6],
            start=True, stop=True,
        )
        C5 = sing.tile([128, 5], BF16, tag="C5")
        C1 = sing.tile([128, 1], BF16, tag="C1")
        nc.vector.tensor_copy(C5[:, 0:1], cp[:, 0:1])                   # c0
        nc.vector.tensor_copy(C5[:, 1:2], cp[:, 513:514])               # c1
        nc.vector.tensor_scalar_mul(C5[:, 2:3], cp[:, 1:2], float(S))   # c2
        nc.vector.tensor_scalar_mul(C5[:, 3:4], cp[:, 512:513], float(S))  # c3
        nc.vector.tensor_copy(C5[:, 4:5], cp[:, 2:3])                   # c4
        nc.vector.tensor_copy(C1[:, 0:1], cp[:, 514:515])               # c5

        # projection: q = C.T @ embT. Pack 16 512-col matmul outputs into one
        # 4-bank psum tile: 4 partition-quadrants (tile_position col 0/32/64/
        # 96) x 4 column slots; then one wide DVE copy and one DMA store per
        # quadrant (4 chunks are DRAM-contiguous).
        def project(tts, C, rows, out_dram, total):
            full, tail = divmod(total, 512)

            def rhs_ap(col0, n):
                j0 = col0 // 2048 * 2048
                return tts[j0][:, col0 - j0 : col0 - j0 + n]

            for c0, ncnk in _chunks(full, 16):
                qp = qpp.tile([128, 2048], F32, tag="q")
                ng = (ncnk + 3) // 4
                for r in range(ncnk):
                    g, j = divmod(r, 4)
                    nc.tensor.matmul(
                        out=qp[32 * g : 32 * g + rows, 512 * j : 512 * (j + 1)],
                        lhsT=C[:, :],
                        rhs=rhs_ap((c0 + r) * 512, 512),
                        start=True,
                        stop=True,
                        tile_position=(0, 32 * g),
                    )
                qs = stg.tile([128, 2048], F32, tag="qs")
                nc.vector.tensor_copy(
                    qs[: 32 * (ng - 1) + rows, :], qp[: 32 * (ng - 1) + rows, :]
                )
                for g in range(ng):
                    w = min(4, ncnk - 4 * g) * 512
                    nc.sync.dma_start(
                        out=out_dram[
                            :, (c0 + 4 * g) * 512 : (c0 + 4 * g) * 512 + w
                        ],
                        in_=qs[32 * g : 32 * g + rows, :w],
                    )
            if tail:
                j = full * 512
                qp = qpp.tile([128, 2048], F32, tag="q")
                nc.tensor.matmul(
                    out=qp[:rows, :tail], lhsT=C[:, :],
                    rhs=rhs_ap(j, tail), start=True, stop=True,
                )
                qs = stg.tile([128, 2048], F32, tag="qs")
                nc.vector.tensor_copy(qs[:rows, :tail], qp[:rows, :tail])
                nc.sync.dma_start(
                    out=out_dram[:, j : j + tail], in_=qs[:rows, :tail]
                )

        project(ttA_s, C5, 5, qA, PLA)
        project(ttB_s, C1, 1, qB, PLB)

    nc.compile()
    return nc


_CACHE = {}


def _programs():
    if "p" not in _CACHE:
        _CACHE["p"] = (build_sums(), build_proj())
    return _CACHE["p"]


# ---------------------------------------------------------------- host glue


def _pack_nodes(arr, tiles):
    """[tiles*128, w] row-major -> [128, tiles*w] with node t*128+p at
    (partition p, cols t*w:(t+1)*w)."""
    w = arr.shape[1]
    return np.ascontiguousarray(
        arr.reshape(tiles, 128, w).transpose(1, 0, 2).reshape(128, tiles * w)
    )


def _build_inputs(emb_A, emb_B, rel_emb, ei0, ei1, nh0, nh1, nt0, nt1):
    import ml_dtypes

    bf16 = ml_dtypes.bfloat16
    A16 = emb_A.astype(bf16)
    B16 = emb_B.astype(bf16)
    AT16 = np.ascontiguousarray(A16.T)
    BT16 = np.ascontiguousarray(B16.T)
    relT = np.ascontiguousarray(rel_emb.transpose(0, 2, 1)).astype(np.float32)

    def counts(idx, n):
        return np.bincount(np.asarray(idx).reshape(-1), minlength=n)

    # count columns: A = (s0, h0, t0, h1); B = (s1, t1)
    cA = np.stack(
        [counts(ei0[1], NA), counts(nh0[:, 0], NA), counts(nt0, NA),
         counts(nh1[:, 0], NA)], axis=1,
    ).astype(bf16)
    cB = np.stack([counts(ei1[1], NB), counts(nt1, NB)], axis=1).astype(bf16)

    in1, in2 = [], []
    for k in range(NCORES):
        a0, b0 = k * SLA, k * SLB
        An = np.zeros((PLA, D), bf16)
        An[:SLA] = A16[a0 : a0 + SLA]
        Bn = np.zeros((PLB, D), bf16)
        Bn[:SLB] = B16[b0 : b0 + SLB]
        ca = np.zeros((PLA, 4), bf16)
        ca[:SLA] = cA[a0 : a0 + SLA]
        cb = np.zeros((PLB, 2), bf16)
        cb[:SLB] = cB[b0 : b0 + SLB]
        At = np.zeros((128, PLA), bf16)
        At[:, :SLA] = AT16[:, a0 : a0 + SLA]
        Bt = np.zeros((128, PLB), bf16)
        Bt[:, :SLB] = BT16[:, b0 : b0 + SLB]
        in1.append(
            {
                "tnA": _pack_nodes(An, TA),
                "tnB": _pack_nodes(Bn, TB),
                "cntA": _pack_nodes(ca, TA),
                "cntB": _pack_nodes(cb, TB),
            }
        )
        in2.append({"ttA": At, "ttB": Bt, "relT": relT})
    return in1, in2


def kernel(
    emb_A,
    emb_B,
    rel_emb,
    edge_index_m0,
    edge_index_m1,
    neg_head_m0,
    neg_head_m1,
    neg_tail_m0,
    neg_tail_m1,
    _results=None,
):
    emb_A = np.ascontiguousarray(np.asarray(emb_A, dtype=np.float32))
    emb_B = np.ascontiguousarray(np.asarray(emb_B, dtype=np.float32))
    rel_emb = np.ascontiguousarray(np.asarray(rel_emb, dtype=np.float32))
    ei0 = np.asarray(edge_index_m0, dtype=np.int64)
    ei1 = np.asarray(edge_index_m1, dtype=np.int64)
    nh0 = np.asarray(neg_head_m0, dtype=np.int64)
    nh1 = np.asarray(neg_head_m1, dtype=np.int64)
    nt0 = np.asarray(neg_tail_m0, dtype=np.int64)
    nt1 = np.asarray(neg_tail_m1, dtype=np.int64)

    p1, p2 = _programs()
    cores = list(range(NCORES))
    in1, in2 = _build_inputs(
        emb_A, emb_B, rel_emb, ei0, ei1, nh0, nh1, nt0, nt1
    )

    r1 = run_bass_kernel_spmd(p1, in1, cores)
    if _results is not None:
        _results.append(r1)
    # unshard the sum-sharded partials: gsum = sum over cores
    gsum = np.sum([r1.results[k]["part"] for k in cores], axis=0)
    gsum = np.ascontiguousarray(gsum.astype(np.float32))
    for m in in2:
        m["gsum"] = gsum

    r2 = run_bass_kernel_spmd(p2, in2, cores)
    if _results is not None:
        _results.append(r2)

    # stitch per-node score slices: q_A [5, NA], q_B [NB]
    q_A = np.concatenate(
        [r2.results[k]["qA"][:, :SLA] for k in cores], axis=1
    )
    q_B = np.concatenate([r2.results[k]["qB"][0, :SLB] for k in cores])

    return np.concatenate(
        [
            q_A[0, ei0[0]],
            q_A[1, ei1[0]],
            q_A[2, nh0.reshape(-1)],
            q_A[3, nh1.reshape(-1)],
            np.repeat(q_A[4, nt0[:, 0]], S),
            np.repeat(q_B[nt1[:, 0]], S),
        ]
    ).astype(np.float32)
